# revision 25
# baseline (speedup 1.0000x reference)
"""CASSI GAP reconstruction (DifferentiableGAPTV) on 8 Trainium2 NeuronCores.

Sharding: H=512 rows -> 8 slabs of 64 output rows, each padded to 128
partition rows with 32-row halos.  dy == 0, so rows couple only through the
depthwise conv row taps (+-2/iter); the halo makes all 12 iterations
collective-free and the cost model charges by free-dim only, so halo rows
are free.

Engine plan per iteration (cost-model balanced):
  DVE : fp16 tensor_tensor muls/adds (2x mode), q/y1 plane ops, colconv for
        a few bands
  Pool: fp16 scalar_tensor_tensor muls/adds for its band share
  PE  : A-phase scatter via fp16 identity matmuls into PSUM; 3x5 conv via
        3 col-tap matmuls whose [128,128] weights carry the full 5-tap row
        conv (banded, edge-masked); rowconv-only matmuls for DVE-colconv
        bands
  ACT : PSUM->SBUF evacuation of conv outputs (fp32->fp16), 3-band groups

The 5x5 Gaussian (sigma=0.5) is separably approximated as (5-tap rows) x
(3-tap cols, renormalized); the dropped +-2 col taps carry 5e-4 of mass.
"""
import sys

sys.path.insert(0, "/opt/trn_rl_repo")
import numpy as np
import concourse.bass as bass
import concourse.mybir as mybir
import concourse.tile as tile
from concourse.ap import AP
from concourse.bass_utils import run_bass_kernel_spmd

H, W, L = 512, 512, 28
N_ITER = 12
SIGMA = 0.5
PI = 3.141592653589793
NCORES = 8
ROWS = 128          # slab rows per core
OUT_ROWS = 64       # exact output rows per core
HALO = 32           # (ROWS - OUT_ROWS) / 2
XP = W + 4          # xs band pitch: 2 zero pad cols each side

f32 = mybir.dt.float32
f16 = mybir.dt.float16

# ---- engine split knobs ----
CGRP = 3             # max bands per conv/evac group (PSUM: 2*CGRP banks + 2 yb)
# conv/evac group sizes; first group small to shorten the q->conv lead-in
GROUPS = [1, 3, 3, 3, 3, 3, 3, 3, 3, 2, 1]
# t/w ops: which groups run on Pool instead of DVE.  Pool is ~3.7x slower
# per element, so it only gets work with far-future deadlines (late conv
# groups, prefetched right after q).
POOL_TW = {7, 8, 9, 10}
# u'-mul (next iteration's m*x): which groups run on Pool
POOL_U = set()
# conv path: groups using 2-matmul conv (DVE tap-sum) instead of 3-matmul
CC2MM = {6, 7, 8, 9, 10}


def _offsets(s, phi_deg):
    phi = phi_deg * PI / 180.0
    dx = s * np.cos(phi)
    dy = s * np.sin(phi)
    dx = dx - dx.min()
    dy = dy - dy.min()
    return np.rint(dx).astype(np.int32), np.rint(dy).astype(np.int32)


def _gauss1d(sigma):
    ksize = max(3, int(6 * sigma + 1) | 1)
    ax = np.arange(ksize, dtype=np.float32) - ksize // 2
    g1 = np.exp(-0.5 * (ax / sigma) ** 2)
    g1 = g1 / g1.sum()
    return g1.astype(np.float32)  # [5]


def _split_excess_waits(nc, max_w=1):
    """walrus accepts at most one sync wait per instruction; hoist excess
    waits onto preceding same-engine NoOp carriers."""
    ctr = 0
    for f in nc.m.functions:
        for bb in f.blocks:
            il = bb.instructions
            i = 0
            while i < len(il):
                inst = il[i]
                si = inst.sync_info
                w = list(si.on_wait) if (si and si.on_wait) else []
                if len(w) > max_w:
                    si.on_wait = w[-max_w:]
                    extra = w[:-max_w]
                    pos = i
                    for j in range(0, len(extra), max_w):
                        ctr += 1
                        nop = mybir.InstNoOp(
                            name=f"I-waitsplit-{ctr}", ins=[], outs=[]
                        )
                        nop.engine = inst.engine
                        nop.sync_info = mybir.SyncInfo(
                            on_wait=extra[j : j + max_w], on_update=[]
                        )
                        il.insert(pos, nop)
                        pos += 1
                        i += 1
                i += 1


def _win3(base_ap, nwin, width):
    """Sliding-window AP: from a [P, width] slice, build [P, nwin, width]
    where window j starts one element after window j-1 (stride 1)."""
    ap = [list(p) for p in base_ap.ap]
    assert len(ap) == 2 and ap[1][0] == 1
    new_ap = [ap[0], [1, nwin], [1, width]]
    return AP(base_ap.tensor, base_ap.offset, new_ap)


def build_nc(dx, n_iter=N_ITER):
    """Build the SPMD Bass program. dx: tuple of L ints (column shifts).
    Requires dx[l] == l (true for the staged problem)."""
    dx = [int(v) for v in dx]
    assert dx == list(range(L)), "kernel assumes dx[l] == l"
    Wm = W + max(dx)     # measurement-plane width (539)
    YBW = W + 28         # even-padded yb width (540) = 2 PSUM banks

    g5 = _gauss1d(SIGMA)             # 5-tap row kernel (exact)
    g3 = g5[1:4] / g5[1:4].sum()     # renormalized 3-tap col kernel
    a3, b3 = float(g3[0]), float(g3[1])

    nc = bass.Bass()
    y_in = nc.declare_dram_parameter("y_slab", [ROWS, Wm], f32, isOutput=False)
    m_in = nc.declare_dram_parameter("m_slab", [ROWS, W], f32, isOutput=False)
    # weights: [I, W_m1, W_0, W_p1, Bd] stacked -> [128, 5, 128] f32 in DRAM
    w_in = nc.declare_dram_parameter("wmats", [128, 5, 128], f32, isOutput=False)
    out = nc.declare_dram_parameter("xout", [L, OUT_ROWS, W], f32, isOutput=True)

    assert sum(GROUPS) == L
    bounds = np.concatenate([[0], np.cumsum(GROUPS)])
    glist = [(g, int(bounds[g]), int(bounds[g + 1])) for g in range(len(GROUPS))]

    with tile.TileContext(nc) as tc:
        with (
            tc.tile_pool(name="state", bufs=1) as st,
            tc.tile_pool(name="ybps", bufs=1, space="PSUM") as ybp,
            tc.tile_pool(name="cps", bufs=2, space="PSUM") as cp,
        ):
            # ---- load inputs ----
            y32 = st.tile([ROWS, Wm], f32)
            m32 = st.tile([ROWS, W], f32)
            w32 = st.tile([128, 5, 128], f32)
            nc.sync.dma_start(y32[:], y_in[:])
            nc.sync.dma_start(m32[:], m_in[:])
            nc.sync.dma_start(w32[:], w_in[:])

            wts = st.tile([128, 5, 128], f16)
            nc.vector.tensor_copy(wts[:], w32[:])
            W_I = wts[:, 0, :]
            W_CC = [wts[:, 1 + j, :] for j in range(3)]   # col-tap x row-conv
            W_BD = wts[:, 4, :]                           # row-conv only (b3 folded)

            m16 = st.tile([ROWS, W], f16)
            nc.vector.tensor_copy(m16[:], m32[:])
            y16 = st.tile([ROWS, Wm], f16)
            nc.vector.tensor_copy(y16[:], y32[:])

            zf16 = st.tile([128, 32], f16)
            nc.vector.memset(zf16[:], 0.0)
            zr16 = st.tile([128, 32], f16)
            nc.vector.memset(zr16[:], 0.0)

            # ---- Phi_sum via PE identity scatter; invPhi = 1/max(Phi,1) ----
            phps = ybp.tile([ROWS, YBW], f32, tag="yb")
            nc.tensor.matmul(phps[:, W:YBW], W_I, zr16[:, :28].to_broadcast((128, 28)),
                             start=True, stop=False, skip_group_check=True)
            for l in range(L):
                nc.tensor.matmul(
                    phps[:, l : l + W], W_I, m16[:],
                    start=(l == 0), stop=(l == L - 1), skip_group_check=True,
                )
            phi32 = st.tile([ROWS, Wm], f32)
            nc.vector.tensor_scalar_max(phi32[:], phps[:, :Wm], 1.0)
            inv_phi = st.tile([ROWS, Wm], f32)
            nc.vector.reciprocal(inv_phi[:], phi32[:])
            ip16 = st.tile([ROWS, Wm], f16)
            nc.vector.tensor_copy(ip16[:], inv_phi[:])
            # mi[l] = m * invPhi[l:l+W]: folds the Phi division into the
            # per-band mask so q = y1py - 2*yb needs no invPhi multiply
            mi = st.tile([ROWS, L, W], f16)
            for g0 in range(0, L, 7):
                g1 = min(g0 + 7, L)
                nc.vector.tensor_mul(
                    out=mi[:, g0:g1, :],
                    in0=m16[:, None, :].to_broadcast((ROWS, g1 - g0, W)),
                    in1=_win3(ip16[:, g0 : g0 + W], g1 - g0, W),
                )

            # ---- state tiles ----
            # xs: x state / conv input w; bands at cols [2, 514), zero pads
            xs = st.tile([ROWS, L, XP], f16)
            nc.vector.memset(xs[:, :, 0:2], 0.0)
            nc.vector.memset(xs[:, :, 2 + W :], 0.0)
            us = st.tile([ROWS, L, W], f16)
            ts = st.tile([ROWS, L, W], f16)
            q16 = st.tile([ROWS, Wm + 5], f16)
            nc.vector.memset(q16[:, Wm:], 0.0)
            qtmp = st.tile([ROWS, Wm], f32)
            y1py = st.tile([ROWS, Wm], f32)
            t1f = st.tile([ROWS, Wm], f32)
            # y1 = y initially -> y1py = y1 + y = 2y
            nc.vector.tensor_scalar_mul(y1py[:], y32[:], 2.0)
            # conv scratch
            va3s = [st.tile([ROWS, CGRP, W], f16, name=f"va3_{i}") for i in range(2)]
            stage = [st.tile([ROWS, CGRP, W], f32, name=f"stg{i}") for i in range(2)]

            # ---- x0 = At(y) = m * y[win];  u0 = m * x0 ----
            for g0 in range(0, L, 7):
                g1 = min(g0 + 7, L)
                n = g1 - g0
                ywin = _win3(y16[:, g0 : g0 + W], n, W)
                nc.vector.tensor_mul(out=xs[:, g0:g1, 2 : 2 + W],
                                     in0=m16[:, None, :].to_broadcast((ROWS, n, W)),
                                     in1=ywin)
            for g, l0, l1 in glist:
                n = l1 - l0
                eng = nc.gpsimd if g in POOL_U else nc.vector
                eng.tensor_mul(
                    out=us[:, l0:l1, :],
                    in0=m16[:, None, :].to_broadcast((ROWS, n, W)),
                    in1=xs[:, l0:l1, 2 : 2 + W],
                )

            # ---- iterations ----
            # u(k) is computed during iteration k-1's conv phase (and in init
            # for k=0).  The A-phase scatter chain for iteration k+1 is
            # interleaved into iteration k's conv phase (lagged 2 groups), so
            # each iteration body starts directly with the q plane ops.
            def emit_A(ybt, l0, l1, first, last_band):
                if first:
                    nc.tensor.matmul(ybt[:, W:YBW], W_I, zr16[:, :28],
                                     start=True, stop=False,
                                     skip_group_check=True)
                for l in range(l0, l1):
                    nc.tensor.matmul(
                        ybt[:, l : l + W], W_I, us[:, l, :],
                        start=(first and l == l0), stop=(l == last_band),
                        skip_group_check=True,
                    )

            # A-chain for iteration 0
            yb = ybp.tile([ROWS, YBW], f32, tag="yb")
            emit_A(yb, 0, L, True, L - 1)

            for it in range(n_iter):
                last = it == n_iter - 1
                # phase B: q = y1py - 2*yb  (fp32 in, fp16 out; invPhi is
                # folded into the per-band mi masks)
                nc.vector.scalar_tensor_tensor(
                    out=q16[:, :Wm], in0=yb[:, :Wm], scalar=-2.0, in1=y1py[:],
                    op0=mybir.AluOpType.mult, op1=mybir.AluOpType.add,
                )

                # phase C: per conv group of CGRP bands:
                #   t = m*q[win]; w = x+t (in xs); conv -> PSUM; evac (ACT);
                #   u'(g) = m*x' right after evac, then (lagged 2 groups) the
                #   next iteration's A-scatter matmuls for those bands.
                if not last:
                    yb_next = ybp.tile([ROWS, YBW], f32, tag="yb")
                a_pending = []
                for g, l0, l1 in glist:
                    n = l1 - l0
                    if g == 2 and not last:
                        # deferred y1py += y - yb, emitted after the first
                        # conv groups so it doesn't delay the w(g0) chain
                        nc.vector.scalar_tensor_tensor(
                            out=t1f[:], in0=yb[:, :Wm], scalar=-1.0, in1=y32[:],
                            op0=mybir.AluOpType.mult, op1=mybir.AluOpType.add,
                        )
                        nc.vector.tensor_add(out=y1py[:], in0=y1py[:], in1=t1f[:])
                    tw = nc.gpsimd if g in POOL_TW else nc.vector
                    qwin = _win3(q16[:, l0 : l0 + W], n, W)
                    tw.tensor_mul(
                        out=ts[:, l0:l1, :], in0=mi[:, l0:l1, :], in1=qwin,
                    )
                    tw.tensor_add(
                        out=xs[:, l0:l1, 2 : 2 + W],
                        in0=xs[:, l0:l1, 2 : 2 + W],
                        in1=ts[:, l0:l1, :],
                    )
                    x2 = cp.tile([ROWS, CGRP, W], f32, tag="x2")
                    if g in CC2MM:
                        # 2-matmul conv: batched tap-sum on DVE, a3*B5 weight
                        va3 = va3s[g % 2]
                        nc.vector.tensor_add(
                            out=va3[:, :n, :], in0=xs[:, l0:l1, 1 : 1 + W],
                            in1=xs[:, l0:l1, 3 : 3 + W],
                        )
                        for j, l in enumerate(range(l0, l1)):
                            nc.tensor.matmul(
                                x2[:, j, :], W_CC[1], xs[:, l, 2 : 2 + W],
                                start=True, stop=False, skip_group_check=True,
                            )
                            nc.tensor.matmul(
                                x2[:, j, :], W_CC[0], va3[:, j, :],
                                start=False, stop=True, skip_group_check=True,
                            )
                    else:
                        # 3-matmul conv: col taps as shifted rhs
                        for j, l in enumerate(range(l0, l1)):
                            for dc in (0, -1, 1):
                                nc.tensor.matmul(
                                    x2[:, j, :], W_CC[dc + 1],
                                    xs[:, l, 2 + dc : 2 + dc + W],
                                    start=(dc == 0), stop=(dc == 1),
                                    skip_group_check=True,
                                )
                    if last:
                        stg = stage[g % 2]
                        nc.scalar.copy(stg[:, :n, :], x2[:, :n, :])
                        for j, l in enumerate(range(l0, l1)):
                            nc.sync.dma_start(
                                out[l, :, :], stg[HALO : HALO + OUT_ROWS, j, :]
                            )
                    else:
                        nc.scalar.copy(xs[:, l0:l1, 2 : 2 + W], x2[:, :n, :])
                        # u' for next iteration (reads evac'd x')
                        ueng = nc.gpsimd if g in POOL_U else nc.vector
                        ueng.tensor_mul(
                            out=us[:, l0:l1, :],
                            in0=m16[:, None, :].to_broadcast((ROWS, n, W)),
                            in1=xs[:, l0:l1, 2 : 2 + W],
                        )
                        a_pending.append((l0, l1))
                        if len(a_pending) > 2:
                            al0, al1 = a_pending.pop(0)
                            emit_A(yb_next, al0, al1, al0 == 0, -1)
                if not last:
                    while a_pending:
                        al0, al1 = a_pending.pop(0)
                        emit_A(yb_next, al0, al1, al0 == 0,
                               L - 1 if al1 == L else -1)
                    yb = yb_next

    _split_excess_waits(nc, max_w=1)
    return nc


def _host_inputs(y_1hw, mask2d, dx):
    """Per-core input maps."""
    y2 = np.asarray(y_1hw, dtype=np.float32)[0]      # [512, Wm]
    m2 = np.asarray(mask2d, dtype=np.float32)        # [512, 512]
    Wm = W + int(max(dx))
    g5 = _gauss1d(SIGMA)
    g3 = g5[1:4] / g5[1:4].sum()
    ident = np.eye(128, dtype=np.float32)

    in_maps = []
    for c in range(NCORES):
        rk = 64 * c - HALO
        y_slab = np.zeros((ROWS, Wm), dtype=np.float32)
        m_slab = np.zeros((ROWS, W), dtype=np.float32)
        lo = max(0, -rk)              # first valid slab row
        hi = min(ROWS, H - rk)        # one past last valid slab row
        y_slab[lo:hi] = y2[rk + lo : rk + hi]
        m_slab[lo:hi] = m2[rk + lo : rk + hi]
        # banded 5-tap row-conv matrix, zeroed outside valid (global) rows
        B5 = np.zeros((128, 128), dtype=np.float32)
        for k in range(-2, 3):
            for i in range(128):
                ip = i + k                      # input slab row
                if lo <= i < hi and lo <= ip < hi:
                    B5[ip, i] = g5[k + 2]
        wm = np.zeros((128, 5, 128), dtype=np.float32)
        wm[:, 0, :] = ident
        for j, cc in enumerate(g3):             # col tap coefficient
            wm[:, 1 + j, :] = cc * B5           # order: [-1? no: j=0->-1]
        # W_CC index mapping: W_CC[dc+1], dc in {-1,0,1} -> j = dc+1 uses g3[dc+1]
        wm[:, 4, :] = g3[1] * B5                # Bd: b3 folded row conv
        in_maps.append({"y_slab": y_slab, "m_slab": m_slab, "wmats": wm})
    return in_maps


_NC_CACHE = {}


def _get_nc(dx, n_iter=N_ITER):
    key = (tuple(int(v) for v in dx), n_iter)
    if key not in _NC_CACHE:
        _NC_CACHE[key] = build_nc(key[0], n_iter)
    return _NC_CACHE[key]


def kernel(y_1hw, mask2d, phi_d_deg, s_nom, n_iter=N_ITER, trace=False):
    s = np.asarray(s_nom, dtype=np.float32)
    phi = float(np.asarray(phi_d_deg))
    dx, dy = _offsets(s, phi)
    assert (dy == 0).all(), "kernel assumes dy == 0"
    nc = _get_nc(dx, n_iter)
    in_maps = _host_inputs(y_1hw, mask2d, dx)
    res = run_bass_kernel_spmd(nc, in_maps, list(range(NCORES)), trace=trace)
    x_full = np.empty((1, L, H, W), dtype=np.float32)
    for c in range(NCORES):
        x_full[0, :, 64 * c : 64 * (c + 1), :] = res.results[c]["xout"]
    kernel.last_results = res
    return x_full


# revision 26
# speedup vs baseline: 1.0241x; 1.0241x over previous
"""CASSI GAP reconstruction (DifferentiableGAPTV) on 8 Trainium2 NeuronCores.

Sharding: H=512 rows -> 8 slabs of 64 output rows, each padded to 128
partition rows with 32-row halos.  dy == 0, so rows couple only through the
depthwise conv row taps (+-2/iter); the halo makes all 12 iterations
collective-free and the cost model charges by free-dim only, so halo rows
are free.

Engine plan per iteration (cost-model balanced):
  DVE : fp16 tensor_tensor muls/adds (2x mode), q/y1 plane ops, colconv for
        a few bands
  Pool: fp16 scalar_tensor_tensor muls/adds for its band share
  PE  : A-phase scatter via fp16 identity matmuls into PSUM; 3x5 conv via
        3 col-tap matmuls whose [128,128] weights carry the full 5-tap row
        conv (banded, edge-masked); rowconv-only matmuls for DVE-colconv
        bands
  ACT : PSUM->SBUF evacuation of conv outputs (fp32->fp16), 3-band groups

The 5x5 Gaussian (sigma=0.5) is separably approximated as (5-tap rows) x
(3-tap cols, renormalized); the dropped +-2 col taps carry 5e-4 of mass.
"""
import sys

sys.path.insert(0, "/opt/trn_rl_repo")
import numpy as np
import concourse.bass as bass
import concourse.mybir as mybir
import concourse.tile as tile
from concourse.ap import AP
from concourse.bass_utils import run_bass_kernel_spmd

H, W, L = 512, 512, 28
N_ITER = 12
SIGMA = 0.5
PI = 3.141592653589793
NCORES = 8
ROWS = 128          # slab rows per core
OUT_ROWS = 64       # exact output rows per core
HALO = 32           # (ROWS - OUT_ROWS) / 2
XP = W + 4          # xs band pitch: 2 zero pad cols each side

f32 = mybir.dt.float32
f16 = mybir.dt.float16

# ---- engine split knobs ----
CGRP = 3             # max bands per conv/evac group (PSUM: 2*CGRP banks + 2 yb)
# conv/evac group sizes; first group small to shorten the q->conv lead-in
GROUPS = [1, 3, 3, 3, 3, 3, 3, 3, 3, 2, 1]
# t/w ops: which groups run on Pool instead of DVE.  Pool is ~3.7x slower
# per element, so it only gets work with far-future deadlines (late conv
# groups, prefetched right after q).
POOL_TW = {7, 8, 9, 10}
# u'-mul (next iteration's m*x): which groups run on Pool
POOL_U = set()
# conv path: groups using 2-matmul conv (DVE tap-sum) instead of 3-matmul
CC2MM = {6, 7, 8, 9, 10}


def _offsets(s, phi_deg):
    phi = phi_deg * PI / 180.0
    dx = s * np.cos(phi)
    dy = s * np.sin(phi)
    dx = dx - dx.min()
    dy = dy - dy.min()
    return np.rint(dx).astype(np.int32), np.rint(dy).astype(np.int32)


def _gauss1d(sigma):
    ksize = max(3, int(6 * sigma + 1) | 1)
    ax = np.arange(ksize, dtype=np.float32) - ksize // 2
    g1 = np.exp(-0.5 * (ax / sigma) ** 2)
    g1 = g1 / g1.sum()
    return g1.astype(np.float32)  # [5]


def _split_excess_waits(nc, max_w=1):
    """walrus accepts at most one sync wait per instruction; hoist excess
    waits onto preceding same-engine NoOp carriers."""
    ctr = 0
    for f in nc.m.functions:
        for bb in f.blocks:
            il = bb.instructions
            i = 0
            while i < len(il):
                inst = il[i]
                si = inst.sync_info
                w = list(si.on_wait) if (si and si.on_wait) else []
                if len(w) > max_w:
                    si.on_wait = w[-max_w:]
                    extra = w[:-max_w]
                    pos = i
                    for j in range(0, len(extra), max_w):
                        ctr += 1
                        nop = mybir.InstNoOp(
                            name=f"I-waitsplit-{ctr}", ins=[], outs=[]
                        )
                        nop.engine = inst.engine
                        nop.sync_info = mybir.SyncInfo(
                            on_wait=extra[j : j + max_w], on_update=[]
                        )
                        il.insert(pos, nop)
                        pos += 1
                        i += 1
                i += 1


def _win3(base_ap, nwin, width):
    """Sliding-window AP: from a [P, width] slice, build [P, nwin, width]
    where window j starts one element after window j-1 (stride 1)."""
    ap = [list(p) for p in base_ap.ap]
    assert len(ap) == 2 and ap[1][0] == 1
    new_ap = [ap[0], [1, nwin], [1, width]]
    return AP(base_ap.tensor, base_ap.offset, new_ap)


def build_nc(dx, n_iter=N_ITER):
    """Build the SPMD Bass program. dx: tuple of L ints (column shifts).
    Requires dx[l] == l (true for the staged problem)."""
    dx = [int(v) for v in dx]
    assert dx == list(range(L)), "kernel assumes dx[l] == l"
    Wm = W + max(dx)     # measurement-plane width (539)
    YBW = W + 28         # even-padded yb width (540) = 2 PSUM banks

    g5 = _gauss1d(SIGMA)             # 5-tap row kernel (exact)
    g3 = g5[1:4] / g5[1:4].sum()     # renormalized 3-tap col kernel
    a3, b3 = float(g3[0]), float(g3[1])

    nc = bass.Bass()
    y_in = nc.declare_dram_parameter("y_slab", [ROWS, Wm], f32, isOutput=False)
    m_in = nc.declare_dram_parameter("m_slab", [ROWS, W], f32, isOutput=False)
    # weights: [I, W_m1, W_0, W_p1, Bd] stacked -> [128, 5, 128] f32 in DRAM
    w_in = nc.declare_dram_parameter("wmats", [128, 5, 128], f32, isOutput=False)
    out = nc.declare_dram_parameter("xout", [L, OUT_ROWS, W], f32, isOutput=True)

    assert sum(GROUPS) == L
    bounds = np.concatenate([[0], np.cumsum(GROUPS)])
    glist = [(g, int(bounds[g]), int(bounds[g + 1])) for g in range(len(GROUPS))]

    with tile.TileContext(nc) as tc:
        with (
            tc.tile_pool(name="state", bufs=1) as st,
            tc.tile_pool(name="ybps", bufs=1, space="PSUM") as ybp,
            tc.tile_pool(name="cps", bufs=2, space="PSUM") as cp,
        ):
            # ---- load inputs ----
            y32 = st.tile([ROWS, Wm], f32)
            m32 = st.tile([ROWS, W], f32)
            w32 = st.tile([128, 5, 128], f32)
            nc.sync.dma_start(y32[:], y_in[:])
            nc.sync.dma_start(m32[:], m_in[:])
            nc.sync.dma_start(w32[:], w_in[:])

            wts = st.tile([128, 5, 128], f16)
            nc.vector.tensor_copy(wts[:], w32[:])
            W_I = wts[:, 0, :]
            W_CC = [wts[:, 1 + j, :] for j in range(3)]   # col-tap x row-conv
            W_BD = wts[:, 4, :]                           # row-conv only (b3 folded)

            m16 = st.tile([ROWS, W], f16)
            nc.vector.tensor_copy(m16[:], m32[:])
            y16 = st.tile([ROWS, Wm], f16)
            nc.vector.tensor_copy(y16[:], y32[:])

            zf16 = st.tile([128, 32], f16)
            nc.vector.memset(zf16[:], 0.0)
            zr16 = st.tile([128, 32], f16)
            nc.vector.memset(zr16[:], 0.0)

            # ---- Phi_sum via PE identity scatter; invPhi = 1/max(Phi,1) ----
            phps = ybp.tile([ROWS, YBW], f32, tag="yb")
            nc.tensor.matmul(phps[:, W:YBW], W_I, zr16[:, :28].to_broadcast((128, 28)),
                             start=True, stop=False, skip_group_check=True)
            for l in range(L):
                nc.tensor.matmul(
                    phps[:, l : l + W], W_I, m16[:],
                    start=(l == 0), stop=(l == L - 1), skip_group_check=True,
                )
            phi32 = st.tile([ROWS, Wm], f32)
            nc.vector.tensor_scalar_max(phi32[:], phps[:, :Wm], 1.0)
            inv_phi = st.tile([ROWS, Wm], f32)
            nc.vector.reciprocal(inv_phi[:], phi32[:])
            ip16 = st.tile([ROWS, Wm], f16)
            nc.vector.tensor_copy(ip16[:], inv_phi[:])
            # mi[l] = m * invPhi[l:l+W]: folds the Phi division into the
            # per-band mask so q = y1py - 2*yb needs no invPhi multiply
            mi = st.tile([ROWS, L, W], f16)
            for g0 in range(0, L, 7):
                g1 = min(g0 + 7, L)
                nc.vector.tensor_mul(
                    out=mi[:, g0:g1, :],
                    in0=m16[:, None, :].to_broadcast((ROWS, g1 - g0, W)),
                    in1=_win3(ip16[:, g0 : g0 + W], g1 - g0, W),
                )

            # ---- state tiles ----
            # xs: x state / conv input w; bands at cols [2, 514), zero pads
            xs = st.tile([ROWS, L, XP], f16)
            nc.vector.memset(xs[:, :, 0:2], 0.0)
            nc.vector.memset(xs[:, :, 2 + W :], 0.0)
            us = st.tile([ROWS, L, W], f16)
            ts = st.tile([ROWS, L, W], f16)
            q16 = st.tile([ROWS, Wm + 5], f16)
            nc.vector.memset(q16[:, Wm:], 0.0)
            qtmp = st.tile([ROWS, Wm], f32)
            y1py = st.tile([ROWS, Wm], f32)
            t1f = st.tile([ROWS, Wm], f32)
            # y1 = y initially -> y1py = y1 + y = 2y
            nc.vector.tensor_scalar_mul(y1py[:], y32[:], 2.0)
            # conv scratch
            va3s = [st.tile([ROWS, CGRP, W], f16, name=f"va3_{i}") for i in range(2)]
            stage = [st.tile([ROWS, CGRP, W], f32, name=f"stg{i}") for i in range(2)]

            # ---- x0 = At(y) = m * y[win];  u0 = m * x0 ----
            for g0 in range(0, L, 7):
                g1 = min(g0 + 7, L)
                n = g1 - g0
                ywin = _win3(y16[:, g0 : g0 + W], n, W)
                nc.vector.tensor_mul(out=xs[:, g0:g1, 2 : 2 + W],
                                     in0=m16[:, None, :].to_broadcast((ROWS, n, W)),
                                     in1=ywin)
            for g, l0, l1 in glist:
                n = l1 - l0
                eng = nc.gpsimd if g in POOL_U else nc.vector
                eng.tensor_mul(
                    out=us[:, l0:l1, :],
                    in0=m16[:, None, :].to_broadcast((ROWS, n, W)),
                    in1=xs[:, l0:l1, 2 : 2 + W],
                )

            # ---- iterations ----
            # u(k) is computed during iteration k-1's conv phase (and in init
            # for k=0).  The A-phase scatter chain for iteration k+1 is
            # interleaved into iteration k's conv phase (lagged 2 groups), so
            # each iteration body starts directly with the q plane ops.
            def emit_A(ybt, l0, l1, first, last_band):
                if first:
                    nc.tensor.matmul(ybt[:, W:YBW], W_I, zr16[:, :28],
                                     start=True, stop=False,
                                     skip_group_check=True)
                for l in range(l0, l1):
                    nc.tensor.matmul(
                        ybt[:, l : l + W], W_I, us[:, l, :],
                        start=(first and l == l0), stop=(l == last_band),
                        skip_group_check=True,
                    )

            # A-chain for iteration 0
            yb = ybp.tile([ROWS, YBW], f32, tag="yb")
            emit_A(yb, 0, L, True, L - 1)

            for it in range(n_iter):
                last = it == n_iter - 1
                # phase B: q = y1py - 2*yb  (fp32 in, fp16 out; invPhi is
                # folded into the per-band mi masks)
                nc.vector.scalar_tensor_tensor(
                    out=q16[:, :Wm], in0=yb[:, :Wm], scalar=-2.0, in1=y1py[:],
                    op0=mybir.AluOpType.mult, op1=mybir.AluOpType.add,
                )

                # phase C: per conv group of CGRP bands:
                #   t = m*q[win]; w = x+t (in xs); conv -> PSUM; evac (ACT);
                #   u'(g) = m*x' right after evac, then (lagged 2 groups) the
                #   next iteration's A-scatter matmuls for those bands.
                if not last:
                    yb_next = ybp.tile([ROWS, YBW], f32, tag="yb")
                a_pending = []
                for g, l0, l1 in glist:
                    n = l1 - l0
                    if g == 2 and not last:
                        # deferred y1py += y - yb, emitted after the first
                        # conv groups so it doesn't delay the w(g0) chain
                        nc.vector.scalar_tensor_tensor(
                            out=t1f[:], in0=yb[:, :Wm], scalar=-1.0, in1=y32[:],
                            op0=mybir.AluOpType.mult, op1=mybir.AluOpType.add,
                        )
                        nc.vector.tensor_add(out=y1py[:], in0=y1py[:], in1=t1f[:])
                    tw = nc.gpsimd if g in POOL_TW else nc.vector
                    qwin = _win3(q16[:, l0 : l0 + W], n, W)
                    tw.tensor_mul(
                        out=ts[:, l0:l1, :], in0=mi[:, l0:l1, :], in1=qwin,
                    )
                    tw.tensor_add(
                        out=xs[:, l0:l1, 2 : 2 + W],
                        in0=xs[:, l0:l1, 2 : 2 + W],
                        in1=ts[:, l0:l1, :],
                    )
                    x2 = cp.tile([ROWS, CGRP, W], f32, tag="x2")
                    if g in CC2MM:
                        # 2-matmul conv: per-band tap-sum on DVE, a3*B5 weight
                        va3 = va3s[g % 2]
                        for j, l in enumerate(range(l0, l1)):
                            nc.vector.tensor_add(
                                out=va3[:, j, :], in0=xs[:, l, 1 : 1 + W],
                                in1=xs[:, l, 3 : 3 + W],
                            )
                            nc.tensor.matmul(
                                x2[:, j, :], W_CC[1], xs[:, l, 2 : 2 + W],
                                start=True, stop=False, skip_group_check=True,
                            )
                            nc.tensor.matmul(
                                x2[:, j, :], W_CC[0], va3[:, j, :],
                                start=False, stop=True, skip_group_check=True,
                            )
                    else:
                        # 3-matmul conv: col taps as shifted rhs
                        for j, l in enumerate(range(l0, l1)):
                            for dc in (0, -1, 1):
                                nc.tensor.matmul(
                                    x2[:, j, :], W_CC[dc + 1],
                                    xs[:, l, 2 + dc : 2 + dc + W],
                                    start=(dc == 0), stop=(dc == 1),
                                    skip_group_check=True,
                                )
                    if last:
                        stg = stage[g % 2]
                        nc.scalar.copy(stg[:, :n, :], x2[:, :n, :])
                        for j, l in enumerate(range(l0, l1)):
                            nc.sync.dma_start(
                                out[l, :, :], stg[HALO : HALO + OUT_ROWS, j, :]
                            )
                    else:
                        nc.scalar.copy(xs[:, l0:l1, 2 : 2 + W], x2[:, :n, :])
                        # u' for next iteration (reads evac'd x')
                        ueng = nc.gpsimd if g in POOL_U else nc.vector
                        ueng.tensor_mul(
                            out=us[:, l0:l1, :],
                            in0=m16[:, None, :].to_broadcast((ROWS, n, W)),
                            in1=xs[:, l0:l1, 2 : 2 + W],
                        )
                        a_pending.append((l0, l1))
                        if len(a_pending) > 2:
                            al0, al1 = a_pending.pop(0)
                            emit_A(yb_next, al0, al1, al0 == 0, -1)
                if not last:
                    while a_pending:
                        al0, al1 = a_pending.pop(0)
                        emit_A(yb_next, al0, al1, al0 == 0,
                               L - 1 if al1 == L else -1)
                    yb = yb_next

    _split_excess_waits(nc, max_w=1)
    return nc


def _host_inputs(y_1hw, mask2d, dx):
    """Per-core input maps."""
    y2 = np.asarray(y_1hw, dtype=np.float32)[0]      # [512, Wm]
    m2 = np.asarray(mask2d, dtype=np.float32)        # [512, 512]
    Wm = W + int(max(dx))
    g5 = _gauss1d(SIGMA)
    g3 = g5[1:4] / g5[1:4].sum()
    ident = np.eye(128, dtype=np.float32)

    in_maps = []
    for c in range(NCORES):
        rk = 64 * c - HALO
        y_slab = np.zeros((ROWS, Wm), dtype=np.float32)
        m_slab = np.zeros((ROWS, W), dtype=np.float32)
        lo = max(0, -rk)              # first valid slab row
        hi = min(ROWS, H - rk)        # one past last valid slab row
        y_slab[lo:hi] = y2[rk + lo : rk + hi]
        m_slab[lo:hi] = m2[rk + lo : rk + hi]
        # banded 5-tap row-conv matrix, zeroed outside valid (global) rows
        B5 = np.zeros((128, 128), dtype=np.float32)
        for k in range(-2, 3):
            for i in range(128):
                ip = i + k                      # input slab row
                if lo <= i < hi and lo <= ip < hi:
                    B5[ip, i] = g5[k + 2]
        wm = np.zeros((128, 5, 128), dtype=np.float32)
        wm[:, 0, :] = ident
        for j, cc in enumerate(g3):             # col tap coefficient
            wm[:, 1 + j, :] = cc * B5           # order: [-1? no: j=0->-1]
        # W_CC index mapping: W_CC[dc+1], dc in {-1,0,1} -> j = dc+1 uses g3[dc+1]
        wm[:, 4, :] = g3[1] * B5                # Bd: b3 folded row conv
        in_maps.append({"y_slab": y_slab, "m_slab": m_slab, "wmats": wm})
    return in_maps


_NC_CACHE = {}


def _get_nc(dx, n_iter=N_ITER):
    key = (tuple(int(v) for v in dx), n_iter)
    if key not in _NC_CACHE:
        _NC_CACHE[key] = build_nc(key[0], n_iter)
    return _NC_CACHE[key]


def kernel(y_1hw, mask2d, phi_d_deg, s_nom, n_iter=N_ITER, trace=False):
    s = np.asarray(s_nom, dtype=np.float32)
    phi = float(np.asarray(phi_d_deg))
    dx, dy = _offsets(s, phi)
    assert (dy == 0).all(), "kernel assumes dy == 0"
    nc = _get_nc(dx, n_iter)
    in_maps = _host_inputs(y_1hw, mask2d, dx)
    res = run_bass_kernel_spmd(nc, in_maps, list(range(NCORES)), trace=trace)
    x_full = np.empty((1, L, H, W), dtype=np.float32)
    for c in range(NCORES):
        x_full[0, :, 64 * c : 64 * (c + 1), :] = res.results[c]["xout"]
    kernel.last_results = res
    return x_full


# revision 29
# speedup vs baseline: 1.0385x; 1.0140x over previous
"""CASSI GAP reconstruction (DifferentiableGAPTV) on 8 Trainium2 NeuronCores.

Sharding: H=512 rows -> 8 slabs of 64 output rows, each padded to 128
partition rows with 32-row halos.  dy == 0, so rows couple only through the
depthwise conv row taps (+-2/iter); the halo makes all 12 iterations
collective-free and the cost model charges by free-dim only, so halo rows
are free.

Engine plan per iteration (cost-model balanced):
  DVE : fp16 tensor_tensor muls/adds (2x mode), q/y1 plane ops, colconv for
        a few bands
  Pool: fp16 scalar_tensor_tensor muls/adds for its band share
  PE  : A-phase scatter via fp16 identity matmuls into PSUM; 3x5 conv via
        3 col-tap matmuls whose [128,128] weights carry the full 5-tap row
        conv (banded, edge-masked); rowconv-only matmuls for DVE-colconv
        bands
  ACT : PSUM->SBUF evacuation of conv outputs (fp32->fp16), 3-band groups

The 5x5 Gaussian (sigma=0.5) is separably approximated as (5-tap rows) x
(3-tap cols, renormalized); the dropped +-2 col taps carry 5e-4 of mass.
"""
import sys

sys.path.insert(0, "/opt/trn_rl_repo")
import numpy as np
import concourse.bass as bass
import concourse.mybir as mybir
import concourse.tile as tile
from concourse.ap import AP
from concourse.bass_utils import run_bass_kernel_spmd

H, W, L = 512, 512, 28
N_ITER = 12
SIGMA = 0.5
PI = 3.141592653589793
NCORES = 8
ROWS = 128          # slab rows per core
OUT_ROWS = 64       # exact output rows per core
HALO = 32           # (ROWS - OUT_ROWS) / 2
XP = W + 4          # xs band pitch: 2 zero pad cols each side

f32 = mybir.dt.float32
f16 = mybir.dt.float16

# ---- engine split knobs ----
CGRP = 3             # max bands per conv/evac group (PSUM: 2*CGRP banks + 2 yb)
# conv/evac group sizes; first group small to shorten the q->conv lead-in
GROUPS = [1, 3, 3, 3, 3, 3, 3, 3, 3, 2, 1]
# t/w ops: which groups run on Pool instead of DVE.  Pool is ~3.7x slower
# per element, so it only gets work with far-future deadlines (late conv
# groups, prefetched right after q).
POOL_TW = {7, 8, 9, 10}
# u'-mul (next iteration's m*x): which groups run on Pool
POOL_U = set()
# conv path: groups using 2-matmul conv (DVE tap-sum) instead of 3-matmul
CC2MM = {6, 7, 8, 9, 10}


def _offsets(s, phi_deg):
    phi = phi_deg * PI / 180.0
    dx = s * np.cos(phi)
    dy = s * np.sin(phi)
    dx = dx - dx.min()
    dy = dy - dy.min()
    return np.rint(dx).astype(np.int32), np.rint(dy).astype(np.int32)


def _gauss1d(sigma):
    ksize = max(3, int(6 * sigma + 1) | 1)
    ax = np.arange(ksize, dtype=np.float32) - ksize // 2
    g1 = np.exp(-0.5 * (ax / sigma) ** 2)
    g1 = g1 / g1.sum()
    return g1.astype(np.float32)  # [5]


def _split_excess_waits(nc, max_w=1):
    """walrus accepts at most one sync wait per instruction; hoist excess
    waits onto preceding same-engine NoOp carriers."""
    ctr = 0
    for f in nc.m.functions:
        for bb in f.blocks:
            il = bb.instructions
            i = 0
            while i < len(il):
                inst = il[i]
                si = inst.sync_info
                w = list(si.on_wait) if (si and si.on_wait) else []
                if len(w) > max_w:
                    si.on_wait = w[-max_w:]
                    extra = w[:-max_w]
                    pos = i
                    for j in range(0, len(extra), max_w):
                        ctr += 1
                        nop = mybir.InstNoOp(
                            name=f"I-waitsplit-{ctr}", ins=[], outs=[]
                        )
                        nop.engine = inst.engine
                        nop.sync_info = mybir.SyncInfo(
                            on_wait=extra[j : j + max_w], on_update=[]
                        )
                        il.insert(pos, nop)
                        pos += 1
                        i += 1
                i += 1


def _win3(base_ap, nwin, width):
    """Sliding-window AP: from a [P, width] slice, build [P, nwin, width]
    where window j starts one element after window j-1 (stride 1)."""
    ap = [list(p) for p in base_ap.ap]
    assert len(ap) == 2 and ap[1][0] == 1
    new_ap = [ap[0], [1, nwin], [1, width]]
    return AP(base_ap.tensor, base_ap.offset, new_ap)


def build_nc(dx, n_iter=N_ITER):
    """Build the SPMD Bass program. dx: tuple of L ints (column shifts).
    Requires dx[l] == l (true for the staged problem)."""
    dx = [int(v) for v in dx]
    assert dx == list(range(L)), "kernel assumes dx[l] == l"
    Wm = W + max(dx)     # measurement-plane width (539)
    YBW = W + 28         # even-padded yb width (540) = 2 PSUM banks

    g5 = _gauss1d(SIGMA)             # 5-tap row kernel (exact)
    g3 = g5[1:4] / g5[1:4].sum()     # renormalized 3-tap col kernel
    a3, b3 = float(g3[0]), float(g3[1])

    nc = bass.Bass()
    y_in = nc.declare_dram_parameter("y_slab", [ROWS, Wm], f32, isOutput=False)
    m_in = nc.declare_dram_parameter("m_slab", [ROWS, W], f32, isOutput=False)
    # weights: [I, W_m1, W_0, W_p1, Bd] stacked -> [128, 5, 128] f32 in DRAM
    w_in = nc.declare_dram_parameter("wmats", [128, 5, 128], f32, isOutput=False)
    out = nc.declare_dram_parameter("xout", [L, OUT_ROWS, W], f32, isOutput=True)

    assert sum(GROUPS) == L
    bounds = np.concatenate([[0], np.cumsum(GROUPS)])
    glist = [(g, int(bounds[g]), int(bounds[g + 1])) for g in range(len(GROUPS))]

    with tile.TileContext(nc) as tc:
        with (
            tc.tile_pool(name="state", bufs=1) as st,
            tc.tile_pool(name="ybps", bufs=1, space="PSUM") as ybp,
            tc.tile_pool(name="cps", bufs=2, space="PSUM") as cp,
        ):
            # ---- load inputs ----
            y32 = st.tile([ROWS, Wm], f32)
            m32 = st.tile([ROWS, W], f32)
            w32 = st.tile([128, 5, 128], f32)
            nc.sync.dma_start(w32[:], w_in[:])
            nc.sync.dma_start(m32[:], m_in[:])
            nc.sync.dma_start(y32[:], y_in[:])

            wts = st.tile([128, 5, 128], f16)
            nc.scalar.copy(wts[:], w32[:])
            W_I = wts[:, 0, :]
            W_CC = [wts[:, 1 + j, :] for j in range(3)]   # col-tap x row-conv
            W_BD = wts[:, 4, :]                           # row-conv only (b3 folded)

            m16 = st.tile([ROWS, W], f16)
            nc.scalar.copy(m16[:], m32[:])
            y16 = st.tile([ROWS, Wm], f16)
            nc.scalar.copy(y16[:], y32[:])

            zf16 = st.tile([128, 32], f16)
            nc.vector.memset(zf16[:], 0.0)
            zr16 = st.tile([128, 32], f16)
            nc.vector.memset(zr16[:], 0.0)

            # ---- Phi_sum via PE identity scatter; invPhi = 1/max(Phi,1) ----
            phps = ybp.tile([ROWS, YBW], f32, tag="yb")
            nc.tensor.matmul(phps[:, W:YBW], W_I, zr16[:, :28].to_broadcast((128, 28)),
                             start=True, stop=False, skip_group_check=True)
            for l in range(L):
                nc.tensor.matmul(
                    phps[:, l : l + W], W_I, m16[:],
                    start=(l == 0), stop=(l == L - 1), skip_group_check=True,
                )
            phi32 = st.tile([ROWS, Wm], f32)
            nc.vector.tensor_scalar_max(phi32[:], phps[:, :Wm], 1.0)
            inv_phi = st.tile([ROWS, Wm], f32)
            nc.vector.reciprocal(inv_phi[:], phi32[:])
            ip16 = st.tile([ROWS, Wm], f16)
            nc.scalar.copy(ip16[:], inv_phi[:])
            # mi[l] = m * invPhi[l:l+W]: folds the Phi division into the
            # per-band mask so q = y1py - 2*yb needs no invPhi multiply
            mi = st.tile([ROWS, L, W], f16)
            for g0 in range(0, L, 7):
                g1 = min(g0 + 7, L)
                nc.vector.tensor_mul(
                    out=mi[:, g0:g1, :],
                    in0=m16[:, None, :].to_broadcast((ROWS, g1 - g0, W)),
                    in1=_win3(ip16[:, g0 : g0 + W], g1 - g0, W),
                )

            # ---- state tiles ----
            # xs: x state / conv input w; bands at cols [2, 514), zero pads
            xs = st.tile([ROWS, L, XP], f16)
            nc.vector.memset(xs[:, :, 0:2], 0.0)
            nc.vector.memset(xs[:, :, 2 + W :], 0.0)
            us = st.tile([ROWS, L, W], f16)
            ts = st.tile([ROWS, L, W], f16)
            q16 = st.tile([ROWS, Wm + 5], f16)
            nc.vector.memset(q16[:, Wm:], 0.0)
            qtmp = st.tile([ROWS, Wm], f32)
            y1py = st.tile([ROWS, Wm], f32)
            t1f = st.tile([ROWS, Wm], f32)
            # y1 = y initially -> y1py = y1 + y = 2y
            nc.vector.tensor_scalar_mul(y1py[:], y32[:], 2.0)
            # conv scratch
            va3s = [st.tile([ROWS, CGRP, W], f16, name=f"va3_{i}") for i in range(2)]
            stage = [st.tile([ROWS, CGRP, W], f32, name=f"stg{i}") for i in range(2)]

            # ---- x0 = At(y) = m * y[win];  u0 = m * x0 ----
            for g0 in range(0, L, 7):
                g1 = min(g0 + 7, L)
                n = g1 - g0
                ywin = _win3(y16[:, g0 : g0 + W], n, W)
                nc.vector.tensor_mul(out=xs[:, g0:g1, 2 : 2 + W],
                                     in0=m16[:, None, :].to_broadcast((ROWS, n, W)),
                                     in1=ywin)
            for g, l0, l1 in glist:
                n = l1 - l0
                eng = nc.gpsimd if g in POOL_U else nc.vector
                eng.tensor_mul(
                    out=us[:, l0:l1, :],
                    in0=m16[:, None, :].to_broadcast((ROWS, n, W)),
                    in1=xs[:, l0:l1, 2 : 2 + W],
                )

            # ---- iterations ----
            # u(k) is computed during iteration k-1's conv phase (and in init
            # for k=0).  The A-phase scatter chain for iteration k+1 is
            # interleaved into iteration k's conv phase (lagged 2 groups), so
            # each iteration body starts directly with the q plane ops.
            def emit_A(ybt, l0, l1, first, last_band):
                if first:
                    nc.tensor.matmul(ybt[:, W:YBW], W_I, zr16[:, :28],
                                     start=True, stop=False,
                                     skip_group_check=True)
                for l in range(l0, l1):
                    nc.tensor.matmul(
                        ybt[:, l : l + W], W_I, us[:, l, :],
                        start=(first and l == l0), stop=(l == last_band),
                        skip_group_check=True,
                    )

            # A-chain for iteration 0
            yb = ybp.tile([ROWS, YBW], f32, tag="yb")
            emit_A(yb, 0, L, True, L - 1)

            for it in range(n_iter):
                last = it == n_iter - 1
                # phase B: q = y1py - 2*yb  (fp32 in, fp16 out; invPhi is
                # folded into the per-band mi masks)
                nc.vector.scalar_tensor_tensor(
                    out=q16[:, :Wm], in0=yb[:, :Wm], scalar=-2.0, in1=y1py[:],
                    op0=mybir.AluOpType.mult, op1=mybir.AluOpType.add,
                )

                # phase C: per conv group of CGRP bands:
                #   t = m*q[win]; w = x+t (in xs); conv -> PSUM; evac (ACT);
                #   u'(g) = m*x' right after evac, then (lagged 2 groups) the
                #   next iteration's A-scatter matmuls for those bands.
                if not last:
                    yb_next = ybp.tile([ROWS, YBW], f32, tag="yb")
                a_pending = []
                for g, l0, l1 in glist:
                    n = l1 - l0
                    if g == 2 and not last:
                        # deferred y1py += y - yb, emitted after the first
                        # conv groups so it doesn't delay the w(g0) chain
                        nc.vector.scalar_tensor_tensor(
                            out=t1f[:], in0=yb[:, :Wm], scalar=-1.0, in1=y32[:],
                            op0=mybir.AluOpType.mult, op1=mybir.AluOpType.add,
                        )
                        nc.vector.tensor_add(out=y1py[:], in0=y1py[:], in1=t1f[:])
                    tw = nc.gpsimd if g in POOL_TW else nc.vector
                    qwin = _win3(q16[:, l0 : l0 + W], n, W)
                    tw.tensor_mul(
                        out=ts[:, l0:l1, :], in0=mi[:, l0:l1, :], in1=qwin,
                    )
                    tw.tensor_add(
                        out=xs[:, l0:l1, 2 : 2 + W],
                        in0=xs[:, l0:l1, 2 : 2 + W],
                        in1=ts[:, l0:l1, :],
                    )
                    x2 = cp.tile([ROWS, CGRP, W], f32, tag="x2")
                    if g in CC2MM:
                        # 2-matmul conv: per-band tap-sum on DVE, a3*B5 weight
                        va3 = va3s[g % 2]
                        for j, l in enumerate(range(l0, l1)):
                            nc.vector.tensor_add(
                                out=va3[:, j, :], in0=xs[:, l, 1 : 1 + W],
                                in1=xs[:, l, 3 : 3 + W],
                            )
                            nc.tensor.matmul(
                                x2[:, j, :], W_CC[1], xs[:, l, 2 : 2 + W],
                                start=True, stop=False, skip_group_check=True,
                            )
                            nc.tensor.matmul(
                                x2[:, j, :], W_CC[0], va3[:, j, :],
                                start=False, stop=True, skip_group_check=True,
                            )
                    else:
                        # 3-matmul conv: col taps as shifted rhs
                        for j, l in enumerate(range(l0, l1)):
                            for dc in (0, -1, 1):
                                nc.tensor.matmul(
                                    x2[:, j, :], W_CC[dc + 1],
                                    xs[:, l, 2 + dc : 2 + dc + W],
                                    start=(dc == 0), stop=(dc == 1),
                                    skip_group_check=True,
                                )
                    if last:
                        stg = stage[g % 2]
                        nc.scalar.copy(stg[:, :n, :], x2[:, :n, :])
                        for j, l in enumerate(range(l0, l1)):
                            nc.sync.dma_start(
                                out[l, :, :], stg[HALO : HALO + OUT_ROWS, j, :]
                            )
                    else:
                        nc.scalar.copy(xs[:, l0:l1, 2 : 2 + W], x2[:, :n, :])
                        # u' for next iteration (reads evac'd x')
                        ueng = nc.gpsimd if g in POOL_U else nc.vector
                        ueng.tensor_mul(
                            out=us[:, l0:l1, :],
                            in0=m16[:, None, :].to_broadcast((ROWS, n, W)),
                            in1=xs[:, l0:l1, 2 : 2 + W],
                        )
                        a_pending.append((l0, l1))
                        if len(a_pending) > 2:
                            al0, al1 = a_pending.pop(0)
                            emit_A(yb_next, al0, al1, al0 == 0, -1)
                if not last:
                    while a_pending:
                        al0, al1 = a_pending.pop(0)
                        emit_A(yb_next, al0, al1, al0 == 0,
                               L - 1 if al1 == L else -1)
                    yb = yb_next

    _split_excess_waits(nc, max_w=1)
    return nc


def _host_inputs(y_1hw, mask2d, dx):
    """Per-core input maps."""
    y2 = np.asarray(y_1hw, dtype=np.float32)[0]      # [512, Wm]
    m2 = np.asarray(mask2d, dtype=np.float32)        # [512, 512]
    Wm = W + int(max(dx))
    g5 = _gauss1d(SIGMA)
    g3 = g5[1:4] / g5[1:4].sum()
    ident = np.eye(128, dtype=np.float32)

    in_maps = []
    for c in range(NCORES):
        rk = 64 * c - HALO
        y_slab = np.zeros((ROWS, Wm), dtype=np.float32)
        m_slab = np.zeros((ROWS, W), dtype=np.float32)
        lo = max(0, -rk)              # first valid slab row
        hi = min(ROWS, H - rk)        # one past last valid slab row
        y_slab[lo:hi] = y2[rk + lo : rk + hi]
        m_slab[lo:hi] = m2[rk + lo : rk + hi]
        # banded 5-tap row-conv matrix, zeroed outside valid (global) rows
        B5 = np.zeros((128, 128), dtype=np.float32)
        for k in range(-2, 3):
            for i in range(128):
                ip = i + k                      # input slab row
                if lo <= i < hi and lo <= ip < hi:
                    B5[ip, i] = g5[k + 2]
        wm = np.zeros((128, 5, 128), dtype=np.float32)
        wm[:, 0, :] = ident
        for j, cc in enumerate(g3):             # col tap coefficient
            wm[:, 1 + j, :] = cc * B5           # order: [-1? no: j=0->-1]
        # W_CC index mapping: W_CC[dc+1], dc in {-1,0,1} -> j = dc+1 uses g3[dc+1]
        wm[:, 4, :] = g3[1] * B5                # Bd: b3 folded row conv
        in_maps.append({"y_slab": y_slab, "m_slab": m_slab, "wmats": wm})
    return in_maps


_NC_CACHE = {}


def _get_nc(dx, n_iter=N_ITER):
    key = (tuple(int(v) for v in dx), n_iter)
    if key not in _NC_CACHE:
        _NC_CACHE[key] = build_nc(key[0], n_iter)
    return _NC_CACHE[key]


def kernel(y_1hw, mask2d, phi_d_deg, s_nom, n_iter=N_ITER, trace=False):
    s = np.asarray(s_nom, dtype=np.float32)
    phi = float(np.asarray(phi_d_deg))
    dx, dy = _offsets(s, phi)
    assert (dy == 0).all(), "kernel assumes dy == 0"
    nc = _get_nc(dx, n_iter)
    in_maps = _host_inputs(y_1hw, mask2d, dx)
    res = run_bass_kernel_spmd(nc, in_maps, list(range(NCORES)), trace=trace)
    x_full = np.empty((1, L, H, W), dtype=np.float32)
    for c in range(NCORES):
        x_full[0, :, 64 * c : 64 * (c + 1), :] = res.results[c]["xout"]
    kernel.last_results = res
    return x_full


# revision 30
# speedup vs baseline: 1.0473x; 1.0085x over previous
"""CASSI GAP reconstruction (DifferentiableGAPTV) on 8 Trainium2 NeuronCores.

Sharding: H=512 rows -> 8 slabs of 64 output rows, each padded to 128
partition rows with 32-row halos.  dy == 0, so rows couple only through the
depthwise conv row taps (+-2/iter); the halo makes all 12 iterations
collective-free and the cost model charges by free-dim only, so halo rows
are free.

Engine plan per iteration (cost-model balanced):
  DVE : fp16 tensor_tensor muls/adds (2x mode), q/y1 plane ops, colconv for
        a few bands
  Pool: fp16 scalar_tensor_tensor muls/adds for its band share
  PE  : A-phase scatter via fp16 identity matmuls into PSUM; 3x5 conv via
        3 col-tap matmuls whose [128,128] weights carry the full 5-tap row
        conv (banded, edge-masked); rowconv-only matmuls for DVE-colconv
        bands
  ACT : PSUM->SBUF evacuation of conv outputs (fp32->fp16), 3-band groups

The 5x5 Gaussian (sigma=0.5) is separably approximated as (5-tap rows) x
(3-tap cols, renormalized); the dropped +-2 col taps carry 5e-4 of mass.
"""
import sys

sys.path.insert(0, "/opt/trn_rl_repo")
import numpy as np
import concourse.bass as bass
import concourse.mybir as mybir
import concourse.tile as tile
from concourse.ap import AP
from concourse.bass_utils import run_bass_kernel_spmd

H, W, L = 512, 512, 28
N_ITER = 12
SIGMA = 0.5
PI = 3.141592653589793
NCORES = 8
ROWS = 128          # slab rows per core
OUT_ROWS = 64       # exact output rows per core
HALO = 32           # (ROWS - OUT_ROWS) / 2
XP = W + 4          # xs band pitch: 2 zero pad cols each side

f32 = mybir.dt.float32
f16 = mybir.dt.float16

# ---- engine split knobs ----
CGRP = 3             # max bands per conv/evac group (PSUM: 2*CGRP banks + 2 yb)
# conv/evac group sizes; first group small to shorten the q->conv lead-in
GROUPS = [1, 3, 3, 3, 3, 3, 3, 3, 3, 2, 1]
# t/w ops: which groups run on Pool instead of DVE.  Pool is ~3.7x slower
# per element, so it only gets work with far-future deadlines (late conv
# groups, prefetched right after q).
POOL_TW = {7, 8, 9, 10}
# u'-mul (next iteration's m*x): which groups run on Pool
POOL_U = set()
# conv path: groups using 2-matmul conv (DVE tap-sum) instead of 3-matmul
CC2MM = {6, 7, 8, 9, 10}


def _offsets(s, phi_deg):
    phi = phi_deg * PI / 180.0
    dx = s * np.cos(phi)
    dy = s * np.sin(phi)
    dx = dx - dx.min()
    dy = dy - dy.min()
    return np.rint(dx).astype(np.int32), np.rint(dy).astype(np.int32)


def _gauss1d(sigma):
    ksize = max(3, int(6 * sigma + 1) | 1)
    ax = np.arange(ksize, dtype=np.float32) - ksize // 2
    g1 = np.exp(-0.5 * (ax / sigma) ** 2)
    g1 = g1 / g1.sum()
    return g1.astype(np.float32)  # [5]


def _split_excess_waits(nc, max_w=1):
    """walrus accepts at most one sync wait per instruction; hoist excess
    waits onto preceding same-engine NoOp carriers."""
    ctr = 0
    for f in nc.m.functions:
        for bb in f.blocks:
            il = bb.instructions
            i = 0
            while i < len(il):
                inst = il[i]
                si = inst.sync_info
                w = list(si.on_wait) if (si and si.on_wait) else []
                if len(w) > max_w:
                    si.on_wait = w[-max_w:]
                    extra = w[:-max_w]
                    pos = i
                    for j in range(0, len(extra), max_w):
                        ctr += 1
                        nop = mybir.InstNoOp(
                            name=f"I-waitsplit-{ctr}", ins=[], outs=[]
                        )
                        nop.engine = inst.engine
                        nop.sync_info = mybir.SyncInfo(
                            on_wait=extra[j : j + max_w], on_update=[]
                        )
                        il.insert(pos, nop)
                        pos += 1
                        i += 1
                i += 1


def _win3(base_ap, nwin, width):
    """Sliding-window AP: from a [P, width] slice, build [P, nwin, width]
    where window j starts one element after window j-1 (stride 1)."""
    ap = [list(p) for p in base_ap.ap]
    assert len(ap) == 2 and ap[1][0] == 1
    new_ap = [ap[0], [1, nwin], [1, width]]
    return AP(base_ap.tensor, base_ap.offset, new_ap)


def build_nc(dx, n_iter=N_ITER):
    """Build the SPMD Bass program. dx: tuple of L ints (column shifts).
    Requires dx[l] == l (true for the staged problem)."""
    dx = [int(v) for v in dx]
    assert dx == list(range(L)), "kernel assumes dx[l] == l"
    Wm = W + max(dx)     # measurement-plane width (539)
    YBW = W + 28         # even-padded yb width (540) = 2 PSUM banks

    g5 = _gauss1d(SIGMA)             # 5-tap row kernel (exact)
    g3 = g5[1:4] / g5[1:4].sum()     # renormalized 3-tap col kernel
    a3, b3 = float(g3[0]), float(g3[1])

    nc = bass.Bass()
    y_in = nc.declare_dram_parameter("y_slab", [ROWS, Wm], f32, isOutput=False)
    m_in = nc.declare_dram_parameter("m_slab", [ROWS, W], f32, isOutput=False)
    # weights: [I, W_m1, W_0, W_p1, Bd] stacked -> [128, 5, 128] f32 in DRAM
    w_in = nc.declare_dram_parameter("wmats", [128, 5, 128], f32, isOutput=False)
    out = nc.declare_dram_parameter("xout", [L, OUT_ROWS, W], f32, isOutput=True)

    assert sum(GROUPS) == L
    bounds = np.concatenate([[0], np.cumsum(GROUPS)])
    glist = [(g, int(bounds[g]), int(bounds[g + 1])) for g in range(len(GROUPS))]

    with tile.TileContext(nc) as tc:
        with (
            tc.tile_pool(name="state", bufs=1) as st,
            tc.tile_pool(name="ybps", bufs=1, space="PSUM") as ybp,
            tc.tile_pool(name="cps", bufs=2, space="PSUM") as cp,
        ):
            # ---- load inputs ----
            y32 = st.tile([ROWS, Wm], f32)
            m32 = st.tile([ROWS, W], f32)
            w32 = st.tile([128, 5, 128], f32)
            nc.sync.dma_start(w32[:], w_in[:])
            nc.sync.dma_start(m32[:], m_in[:])
            nc.sync.dma_start(y32[:], y_in[:])

            wts = st.tile([128, 5, 128], f16)
            nc.scalar.copy(wts[:], w32[:])
            W_I = wts[:, 0, :]
            W_CC = [wts[:, 1 + j, :] for j in range(3)]   # col-tap x row-conv
            W_BD = wts[:, 4, :]                           # row-conv only (b3 folded)

            m16 = st.tile([ROWS, W], f16)
            nc.scalar.copy(m16[:], m32[:])
            y16 = st.tile([ROWS, Wm], f16)
            nc.scalar.copy(y16[:], y32[:])

            zf16 = st.tile([128, 32], f16)
            nc.vector.memset(zf16[:], 0.0)
            zr16 = st.tile([128, 32], f16)
            nc.vector.memset(zr16[:], 0.0)

            # ---- PE p-state warmup: tiny dependency-free matmuls burn
            # through the 3us ramp window at ~25ns each so the real chains
            # below dispatch at the full 2.4GHz rate ----
            phps = ybp.tile([ROWS, YBW], f32, tag="yb")
            for i in range(140):
                nc.tensor.matmul(
                    phps[0:16, 0:16], zr16[:, :16], zr16[:, :16],
                    start=True, stop=(i == 139), skip_group_check=True,
                )

            # ---- Phi_sum via PE identity scatter; invPhi = 1/max(Phi,1) ----
            nc.tensor.matmul(phps[:, W:YBW], W_I, zr16[:, :28].to_broadcast((128, 28)),
                             start=True, stop=False, skip_group_check=True)
            for l in range(L):
                nc.tensor.matmul(
                    phps[:, l : l + W], W_I, m16[:],
                    start=(l == 0), stop=(l == L - 1), skip_group_check=True,
                )
            phi32 = st.tile([ROWS, Wm], f32)
            nc.vector.tensor_scalar_max(phi32[:], phps[:, :Wm], 1.0)
            inv_phi = st.tile([ROWS, Wm], f32)
            nc.vector.reciprocal(inv_phi[:], phi32[:])
            ip16 = st.tile([ROWS, Wm], f16)
            nc.scalar.copy(ip16[:], inv_phi[:])
            # mi[l] = m * invPhi[l:l+W]: folds the Phi division into the
            # per-band mask so q = y1py - 2*yb needs no invPhi multiply
            mi = st.tile([ROWS, L, W], f16)
            for g0 in range(0, L, 7):
                g1 = min(g0 + 7, L)
                nc.vector.tensor_mul(
                    out=mi[:, g0:g1, :],
                    in0=m16[:, None, :].to_broadcast((ROWS, g1 - g0, W)),
                    in1=_win3(ip16[:, g0 : g0 + W], g1 - g0, W),
                )

            # ---- state tiles ----
            # xs: x state / conv input w; bands at cols [2, 514), zero pads
            xs = st.tile([ROWS, L, XP], f16)
            nc.vector.memset(xs[:, :, 0:2], 0.0)
            nc.vector.memset(xs[:, :, 2 + W :], 0.0)
            us = st.tile([ROWS, L, W], f16)
            ts = st.tile([ROWS, L, W], f16)
            q16 = st.tile([ROWS, Wm + 5], f16)
            nc.vector.memset(q16[:, Wm:], 0.0)
            qtmp = st.tile([ROWS, Wm], f32)
            y1py = st.tile([ROWS, Wm], f32)
            t1f = st.tile([ROWS, Wm], f32)
            # y1 = y initially -> y1py = y1 + y = 2y
            nc.vector.tensor_scalar_mul(y1py[:], y32[:], 2.0)
            # conv scratch
            va3s = [st.tile([ROWS, CGRP, W], f16, name=f"va3_{i}") for i in range(2)]
            stage = [st.tile([ROWS, CGRP, W], f32, name=f"stg{i}") for i in range(2)]

            # ---- x0 = At(y) = m * y[win];  u0 = m * x0 ----
            for g0 in range(0, L, 7):
                g1 = min(g0 + 7, L)
                n = g1 - g0
                ywin = _win3(y16[:, g0 : g0 + W], n, W)
                nc.vector.tensor_mul(out=xs[:, g0:g1, 2 : 2 + W],
                                     in0=m16[:, None, :].to_broadcast((ROWS, n, W)),
                                     in1=ywin)
            for g, l0, l1 in glist:
                n = l1 - l0
                eng = nc.gpsimd if g in POOL_U else nc.vector
                eng.tensor_mul(
                    out=us[:, l0:l1, :],
                    in0=m16[:, None, :].to_broadcast((ROWS, n, W)),
                    in1=xs[:, l0:l1, 2 : 2 + W],
                )

            # ---- iterations ----
            # u(k) is computed during iteration k-1's conv phase (and in init
            # for k=0).  The A-phase scatter chain for iteration k+1 is
            # interleaved into iteration k's conv phase (lagged 2 groups), so
            # each iteration body starts directly with the q plane ops.
            def emit_A(ybt, l0, l1, first, last_band):
                if first:
                    nc.tensor.matmul(ybt[:, W:YBW], W_I, zr16[:, :28],
                                     start=True, stop=False,
                                     skip_group_check=True)
                for l in range(l0, l1):
                    nc.tensor.matmul(
                        ybt[:, l : l + W], W_I, us[:, l, :],
                        start=(first and l == l0), stop=(l == last_band),
                        skip_group_check=True,
                    )

            # A-chain for iteration 0
            yb = ybp.tile([ROWS, YBW], f32, tag="yb")
            emit_A(yb, 0, L, True, L - 1)

            for it in range(n_iter):
                last = it == n_iter - 1
                # phase B: q = y1py - 2*yb  (fp32 in, fp16 out; invPhi is
                # folded into the per-band mi masks)
                nc.vector.scalar_tensor_tensor(
                    out=q16[:, :Wm], in0=yb[:, :Wm], scalar=-2.0, in1=y1py[:],
                    op0=mybir.AluOpType.mult, op1=mybir.AluOpType.add,
                )

                # phase C: per conv group of CGRP bands:
                #   t = m*q[win]; w = x+t (in xs); conv -> PSUM; evac (ACT);
                #   u'(g) = m*x' right after evac, then (lagged 2 groups) the
                #   next iteration's A-scatter matmuls for those bands.
                if not last:
                    yb_next = ybp.tile([ROWS, YBW], f32, tag="yb")
                a_pending = []
                for g, l0, l1 in glist:
                    n = l1 - l0
                    if g == 2 and not last:
                        # deferred y1py += y - yb, emitted after the first
                        # conv groups so it doesn't delay the w(g0) chain
                        nc.vector.scalar_tensor_tensor(
                            out=t1f[:], in0=yb[:, :Wm], scalar=-1.0, in1=y32[:],
                            op0=mybir.AluOpType.mult, op1=mybir.AluOpType.add,
                        )
                        nc.vector.tensor_add(out=y1py[:], in0=y1py[:], in1=t1f[:])
                    tw = nc.gpsimd if g in POOL_TW else nc.vector
                    qwin = _win3(q16[:, l0 : l0 + W], n, W)
                    tw.tensor_mul(
                        out=ts[:, l0:l1, :], in0=mi[:, l0:l1, :], in1=qwin,
                    )
                    tw.tensor_add(
                        out=xs[:, l0:l1, 2 : 2 + W],
                        in0=xs[:, l0:l1, 2 : 2 + W],
                        in1=ts[:, l0:l1, :],
                    )
                    x2 = cp.tile([ROWS, CGRP, W], f32, tag="x2")
                    if g in CC2MM:
                        # 2-matmul conv: per-band tap-sum on DVE, a3*B5 weight
                        va3 = va3s[g % 2]
                        for j, l in enumerate(range(l0, l1)):
                            nc.vector.tensor_add(
                                out=va3[:, j, :], in0=xs[:, l, 1 : 1 + W],
                                in1=xs[:, l, 3 : 3 + W],
                            )
                            nc.tensor.matmul(
                                x2[:, j, :], W_CC[1], xs[:, l, 2 : 2 + W],
                                start=True, stop=False, skip_group_check=True,
                            )
                            nc.tensor.matmul(
                                x2[:, j, :], W_CC[0], va3[:, j, :],
                                start=False, stop=True, skip_group_check=True,
                            )
                    else:
                        # 3-matmul conv: col taps as shifted rhs
                        for j, l in enumerate(range(l0, l1)):
                            for dc in (0, -1, 1):
                                nc.tensor.matmul(
                                    x2[:, j, :], W_CC[dc + 1],
                                    xs[:, l, 2 + dc : 2 + dc + W],
                                    start=(dc == 0), stop=(dc == 1),
                                    skip_group_check=True,
                                )
                    if last:
                        stg = stage[g % 2]
                        nc.scalar.copy(stg[:, :n, :], x2[:, :n, :])
                        for j, l in enumerate(range(l0, l1)):
                            nc.sync.dma_start(
                                out[l, :, :], stg[HALO : HALO + OUT_ROWS, j, :]
                            )
                    else:
                        nc.scalar.copy(xs[:, l0:l1, 2 : 2 + W], x2[:, :n, :])
                        # u' for next iteration (reads evac'd x')
                        ueng = nc.gpsimd if g in POOL_U else nc.vector
                        ueng.tensor_mul(
                            out=us[:, l0:l1, :],
                            in0=m16[:, None, :].to_broadcast((ROWS, n, W)),
                            in1=xs[:, l0:l1, 2 : 2 + W],
                        )
                        a_pending.append((l0, l1))
                        if len(a_pending) > 2:
                            al0, al1 = a_pending.pop(0)
                            emit_A(yb_next, al0, al1, al0 == 0, -1)
                if not last:
                    while a_pending:
                        al0, al1 = a_pending.pop(0)
                        emit_A(yb_next, al0, al1, al0 == 0,
                               L - 1 if al1 == L else -1)
                    yb = yb_next

    _split_excess_waits(nc, max_w=1)
    return nc


def _host_inputs(y_1hw, mask2d, dx):
    """Per-core input maps."""
    y2 = np.asarray(y_1hw, dtype=np.float32)[0]      # [512, Wm]
    m2 = np.asarray(mask2d, dtype=np.float32)        # [512, 512]
    Wm = W + int(max(dx))
    g5 = _gauss1d(SIGMA)
    g3 = g5[1:4] / g5[1:4].sum()
    ident = np.eye(128, dtype=np.float32)

    in_maps = []
    for c in range(NCORES):
        rk = 64 * c - HALO
        y_slab = np.zeros((ROWS, Wm), dtype=np.float32)
        m_slab = np.zeros((ROWS, W), dtype=np.float32)
        lo = max(0, -rk)              # first valid slab row
        hi = min(ROWS, H - rk)        # one past last valid slab row
        y_slab[lo:hi] = y2[rk + lo : rk + hi]
        m_slab[lo:hi] = m2[rk + lo : rk + hi]
        # banded 5-tap row-conv matrix, zeroed outside valid (global) rows
        B5 = np.zeros((128, 128), dtype=np.float32)
        for k in range(-2, 3):
            for i in range(128):
                ip = i + k                      # input slab row
                if lo <= i < hi and lo <= ip < hi:
                    B5[ip, i] = g5[k + 2]
        wm = np.zeros((128, 5, 128), dtype=np.float32)
        wm[:, 0, :] = ident
        for j, cc in enumerate(g3):             # col tap coefficient
            wm[:, 1 + j, :] = cc * B5           # order: [-1? no: j=0->-1]
        # W_CC index mapping: W_CC[dc+1], dc in {-1,0,1} -> j = dc+1 uses g3[dc+1]
        wm[:, 4, :] = g3[1] * B5                # Bd: b3 folded row conv
        in_maps.append({"y_slab": y_slab, "m_slab": m_slab, "wmats": wm})
    return in_maps


_NC_CACHE = {}


def _get_nc(dx, n_iter=N_ITER):
    key = (tuple(int(v) for v in dx), n_iter)
    if key not in _NC_CACHE:
        _NC_CACHE[key] = build_nc(key[0], n_iter)
    return _NC_CACHE[key]


def kernel(y_1hw, mask2d, phi_d_deg, s_nom, n_iter=N_ITER, trace=False):
    s = np.asarray(s_nom, dtype=np.float32)
    phi = float(np.asarray(phi_d_deg))
    dx, dy = _offsets(s, phi)
    assert (dy == 0).all(), "kernel assumes dy == 0"
    nc = _get_nc(dx, n_iter)
    in_maps = _host_inputs(y_1hw, mask2d, dx)
    res = run_bass_kernel_spmd(nc, in_maps, list(range(NCORES)), trace=trace)
    x_full = np.empty((1, L, H, W), dtype=np.float32)
    for c in range(NCORES):
        x_full[0, :, 64 * c : 64 * (c + 1), :] = res.results[c]["xout"]
    kernel.last_results = res
    return x_full
